# revision 30
# baseline (speedup 1.0000x reference)
"""Trainium2 Bass kernel for nn_DownBlock (PacConv1x1 -> PReLU -> Conv6x6s2 -> PReLU).

Math notes:
  - The PacConv2d adaptive kernel is exp(-0.5*||g-g||^2) == 1 exactly, so the
    guide tensor is mathematically unused: stage 1 is a plain 1x1 conv.
  - Stage 1: h[f,y,x] = prelu(sum_c pac_w[f,c] * x[c,y,x] + pac_b[f], alpha1)
  - Stage 2: 6x6 stride-2 conv with padding 2, + bias, prelu.

Implementation (per core, 2 of the 16 batch images, data-parallel over batch):
  - Stage 1 is a K=128 float32r matmul per 512-position tile; the rhs access
    pattern picks x-parity phases so the epilogue (bias+prelu, split between
    ACT and DVE) writes directly into the stage-2 input layout:
        Hx[(px, f), y+2, x//2 + 1]   (128 partitions, 132x66 image, zero halo)
  - Stage 2 (stride-2 6x6 conv) contracts (px, c) = 128 partitions per tap:
        out[o, i, j] = sum_{ky, n} Wp[ky,n][(px,c), o].T @ Hx[:, 2i+ky, j+n]
    = 18 accumulating K=128/M=64/N=512 matmuls per 8-row output block, then a
    fused Prelu epilogue and a DMA out.
  - S2_BF16 selects the stage-2 operand dtype: float32r (TF32-class accuracy,
    M=64 fills the whole PE array) or bfloat16 (two blocks run concurrently in
    the two column halves of the PE array via tile_position -> ~2x stage-2
    matmul throughput at bf16 input rounding).
"""
import numpy as np

import concourse.bacc as bacc
import concourse.mybir as mybir
from concourse.tile import TileContext
from concourse.bass_utils import run_bass_kernel_spmd
from concourse.masks import make_identity

F32 = mybir.dt.float32
F32R = mybir.dt.float32r
BF16 = mybir.dt.bfloat16
FP16 = mybir.dt.float16
ALU = mybir.AluOpType

N_CORES = 8
B_TOTAL = 16
B_PER_CORE = B_TOTAL // N_CORES  # 2
CIN = 128
F = 64   # intermediate / output channels
H = W = 128
HO = WO = 64
K = 6
# phase image: rows 0..131 (y+2), cols 0..65 (x//2+1), zero halo
PR = 132
PC = 66

S2_BF16 = True  # stage-2 matmul dtype: False -> float32r, True -> float16

_CACHE = {}


def _build(repeat=1, s2_bf16=S2_BF16):
    """Build the Bass module.  repeat>1 re-emits the main pipeline that many
    times back-to-back (bench-only: lets wall-clock slope isolate per-pass
    device time from the multi-ms axon dispatch overhead)."""
    nc = bacc.Bacc("TRN2", target_bir_lowering=False, debug=False)

    DT2 = FP16 if s2_bf16 else F32R
    # weights arrive pre-transposed / pre-cast from the host (numpy prep in
    # kernel()): pac_wT[c, f] = pac_w[f, c];  wp[(px, c), ky*3+n, o] =
    # conv_w[o, c, ky, 2n+px] in the stage-2 dtype.
    x = nc.declare_dram_parameter("x", [B_PER_CORE, CIN, H, W], DT2, isOutput=False)
    pac_wT_in = nc.declare_dram_parameter("pac_wT", [CIN, F], DT2, isOutput=False)
    pac_b = nc.declare_dram_parameter("pac_b", [F], F32, isOutput=False)
    alpha1 = nc.declare_dram_parameter("alpha1", [1], F32, isOutput=False)
    wp_in = nc.declare_dram_parameter("wp", [CIN, 18 * F], DT2, isOutput=False)
    conv_b = nc.declare_dram_parameter("conv_b", [F], F32, isOutput=False)
    alpha2 = nc.declare_dram_parameter("alpha2", [1], F32, isOutput=False)
    out = nc.declare_dram_parameter("out", [B_PER_CORE, F, HO, WO], F32, isOutput=True)

    PRELU = mybir.ActivationFunctionType.Prelu

    with TileContext(nc) as tc:
        with (
            tc.tile_pool(name="const", bufs=1) as const,
            tc.tile_pool(name="xin", bufs=8) as xin,
            tc.tile_pool(name="hx", bufs=1) as hxp,
            tc.tile_pool(name="ob", bufs=4) as obp,
            tc.tile_pool(name="dv", bufs=4) as dvp,
            tc.tile_pool(name="psA", bufs=4, space="PSUM") as psA,
        ):
            # ---------------- constants ----------------
            # per-partition scalars; stage-1 reads [0:64], the fp16 stage-2
            # epilogue reads all 128 (conv_b duplicated in both halves)
            b1 = const.tile([CIN, 1], F32)
            b2 = const.tile([CIN, 1], F32)
            a1 = const.tile([CIN, 1], F32)
            a2 = const.tile([CIN, 1], F32)
            nc.sync.dma_start(out=b1[0:64, :], in_=pac_b[:, None])
            nc.sync.dma_start(out=b1[64:128, :], in_=pac_b[:, None])
            nc.sync.dma_start(out=b2[0:64, :], in_=conv_b[:, None])
            nc.sync.dma_start(out=b2[64:128, :], in_=conv_b[:, None])
            nc.sync.dma_start(out=a1[:], in_=alpha1.broadcast_to([CIN, 1]))
            nc.sync.dma_start(out=a2[:], in_=alpha2.broadcast_to([CIN, 1]))

            pac_wT = const.tile([CIN, F], DT2)
            if DT2 == F32R:
                nc.sync.dma_start(out=pac_wT[:], in_=pac_wT_in[:].bitcast(F32R))
            else:
                nc.sync.dma_start(out=pac_wT[:], in_=pac_wT_in[:])
            wp = const.tile([CIN, 18 * F], DT2)
            if DT2 == F32R:
                nc.sync.dma_start(out=wp[:], in_=wp_in[:].bitcast(F32R))
            else:
                nc.sync.dma_start(out=wp[:], in_=wp_in[:])

            # ---------------- phase tensors + halo zeroing ----------------
            zrow = const.tile([CIN, 2 * PC], F32)
            nc.gpsimd.memset(zrow[:], 0.0)

            hx = [
                hxp.tile([CIN, PR * PC], DT2, tag=f"hx{b}", name=f"hx{b}")
                for b in range(B_PER_CORE)
            ]
            for b in range(B_PER_CORE):
                t = hx[b]
                # top rows 0..1, bottom rows 130..131
                nc.vector.tensor_copy(t[:, 0:2 * PC], zrow[:])
                nc.vector.tensor_copy(t[:, 130 * PC:132 * PC], zrow[:])
                # left col 0 and right col 65 stripes (132 rows each)
                nc.vector.tensor_copy(t[:, 0:PR * PC:PC], zrow[:, 0:PR])
                nc.vector.tensor_copy(t[:, 65:PR * PC:PC], zrow[:, 0:PR])

            # ---------------- main pipeline ----------------
            for b in [bb for _ in range(repeat) for bb in range(B_PER_CORE)]:
                hxb = hx[b].rearrange("p (r c) -> p r c", c=PC)
                for t in range(8):  # 16-row x chunks
                    xt = xin.tile([CIN, 16 * W], DT2, tag="xt")
                    xtv = xt[:].rearrange("p (r c) -> p r c", r=16)
                    # two half-chunk DMAs: h3=0 matmuls start after only 8 rows
                    nc.sync.dma_start(
                        out=xtv[:, 0:8],
                        in_=x[b, :, 16 * t:16 * t + 8, :]
                        if DT2 != F32R else x[b, :, 16 * t:16 * t + 8, :].bitcast(F32R),
                    )
                    nc.sync.dma_start(
                        out=xtv[:, 8:16],
                        in_=x[b, :, 16 * t + 8:16 * t + 16, :]
                        if DT2 != F32R else x[b, :, 16 * t + 8:16 * t + 16, :].bitcast(F32R),
                    )
                    for h3 in range(2):  # 8-row halves
                        r0 = 16 * t + 8 * h3 + 2
                        if s2_bf16:
                            # both x-parity halves into one bank (each matmul is
                            # a closed start+stop group, so no pending-group
                            # conflict), then ONE full-width prelu epilogue.
                            ps = psA.tile([CIN, 8, 64], F32, tag="s1", bufs=4,
                                          name="ps")
                            for px in range(2):
                                nc.tensor.matmul(
                                    ps[px * 64:(px + 1) * 64],
                                    pac_wT[:],
                                    xtv[:, 8 * h3:8 * h3 + 8, px::2],
                                    start=True, stop=True,
                                    tile_position=(0, px * 64),
                                )
                            dst = hxb[0:128, r0:r0 + 8, 1:65]
                            if h3 == 0 and t % 2 == 0:
                                # keep some epilogue work on DVE for balance
                                t1 = dvp.tile([CIN, 8, 64], F32, tag="dv1", name="t1")
                                t2 = dvp.tile([CIN, 8, 64], F32, tag="dv2", name="t2")
                                nc.vector.tensor_scalar(
                                    t1[:], ps[:], b1[:], 0.0, ALU.add, ALU.max)
                                nc.vector.tensor_scalar(
                                    t2[:], ps[:], b1[:], 0.0, ALU.add, ALU.min)
                                nc.vector.scalar_tensor_tensor(
                                    dst, t2[:], a1[:], t1[:], ALU.mult, ALU.add)
                            else:
                                nc.scalar.activation(
                                    dst, ps[:], PRELU,
                                    bias=b1[:], scale=1.0, alpha=a1[:],
                                )
                        else:
                            for px in range(2):
                                ps = psA.tile([F, 8, 64], F32, tag="s1")
                                nc.tensor.matmul(
                                    ps[:],
                                    pac_wT[:],
                                    xtv[:, 8 * h3:8 * h3 + 8, px::2],
                                    start=True, stop=True,
                                )
                                dst = hxb[px * 64:(px + 1) * 64, r0:r0 + 8, 1:65]
                                nc.scalar.activation(
                                    dst, ps[:], PRELU,
                                    bias=b1[0:64, :], scale=1.0, alpha=a1[0:64, :],
                                )
                    if s2_bf16:
                        # paired blocks: pair s ready after chunk 2s+2
                        if t >= 2 and t % 2 == 0:
                            _s2_pair(nc, psA, obp, hxb, wp, b2, a2, out, b, (t - 2) // 2)
                    else:
                        if t >= 1:
                            _s2_single(nc, psA, obp, hxb, wp, b2, a2, out, b, t - 1)
                if s2_bf16:
                    _s2_pair(nc, psA, obp, hxb, wp, b2, a2, out, b, 3)
                else:
                    _s2_single(nc, psA, obp, hxb, wp, b2, a2, out, b, 7)

    nc.compile()
    return nc


def _s2_single(nc, psA, obp, hxb, wp, b2, a2, out, b, ib):
    """float32r: 18 accumulating taps -> prelu -> dma, output rows [8ib, 8ib+8)."""
    PRELU = mybir.ActivationFunctionType.Prelu
    ps = psA.tile([F, 8, 64], F32, tag="s2", name="ps")
    for ky in range(K):
        for n in range(3):
            t18 = ky * 3 + n
            r0 = 16 * ib + ky
            rhs = hxb[:, r0:min(r0 + 16, PR):2, n:n + 64]
            nc.tensor.matmul(
                ps[:], wp[:, t18 * F:(t18 + 1) * F], rhs,
                start=(t18 == 0), stop=(t18 == 17),
            )
    ot = obp.tile([F, 8, 64], F32, tag="ot", name="ot")
    nc.scalar.activation(ot[:], ps[:], PRELU, bias=b2[0:64, :], scale=1.0,
                         alpha=a2[0:64, :])
    nc.sync.dma_start(out=out[b, :, 8 * ib:8 * ib + 8, :], in_=ot[:])


def _s2_pair(nc, psA, obp, hxb, wp, b2, a2, out, b, s):
    """bf16/fp16: blocks 2s and 2s+1 run concurrently in the two column halves
    of the PE array (tile_position).  Each half accumulates in its OWN psum
    bank — two interleaved accumulation groups in one bank are illegal (the
    sequencer tracks pending groups per bank zero-region)."""
    PRELU = mybir.ActivationFunctionType.Prelu
    psh = [
        psA.tile([CIN, 8, 64], F32, tag=f"s2{'ab'[h]}", name=f"ps{h}", bufs=2)
        for h in range(2)
    ]
    for ky in range(K):
        for n in range(3):
            t18 = ky * 3 + n
            for half in range(2):
                ib = 2 * s + half
                r0 = 16 * ib + ky
                rhs = hxb[:, r0:min(r0 + 16, PR):2, n:n + 64]
                nc.tensor.matmul(
                    psh[half][half * 64:(half + 1) * 64],
                    wp[:, t18 * F:(t18 + 1) * F],
                    rhs, start=(t18 == 0), stop=(t18 == 17),
                    tile_position=(0, half * 64),
                )
    ot = obp.tile([CIN, 8, 64], F32, tag="ot", name="ot")
    for half in range(2):
        sl = slice(half * 64, (half + 1) * 64)
        nc.scalar.activation(ot[sl], psh[half][sl], PRELU,
                             bias=b2[sl, :], scale=1.0, alpha=a2[sl, :])
    for half in range(2):
        ib = 2 * s + half
        nc.sync.dma_start(
            out=out[b, :, 8 * ib:8 * ib + 8, :],
            in_=ot[half * 64:(half + 1) * 64],
        )


def _get_nc(repeat=1, s2_bf16=S2_BF16):
    key = f"nc{repeat}_{s2_bf16}"
    if key not in _CACHE:
        _CACHE[key] = _build(repeat, s2_bf16)
    return _CACHE[key]


def make_in_maps(x, pac_w, pac_b, alpha1, conv_w, conv_b, alpha2):
    """Host-side prep: shard x over cores, pre-transpose/cast the weights."""
    np_dtx = np.float16 if S2_BF16 else np.float32
    x = np.ascontiguousarray(np.asarray(x, dtype=np.float32), dtype=np_dtx)
    pac_w = np.asarray(pac_w, dtype=np.float32).reshape(F, CIN)
    conv_w4 = np.asarray(conv_w, dtype=np.float32).reshape(F, F, K, K)
    np_dt2 = np.float16 if S2_BF16 else np.float32
    wp_np = np.zeros((CIN, 18 * F), dtype=np_dt2)
    for ky in range(K):
        for n in range(3):
            t18 = ky * 3 + n
            for px in range(2):
                # wp[(px, c), t18*F + o] = conv_w[o, c, ky, 2n+px]
                wp_np[px * F:(px + 1) * F, t18 * F:(t18 + 1) * F] = (
                    conv_w4[:, :, ky, 2 * n + px].T.astype(np_dt2)
                )
    shared = {
        "pac_wT": np.ascontiguousarray(pac_w.T.astype(np_dtx)),
        "pac_b": np.ascontiguousarray(pac_b, dtype=np.float32),
        "alpha1": np.ascontiguousarray(alpha1, dtype=np.float32),
        "wp": wp_np,
        "conv_b": np.ascontiguousarray(conv_b, dtype=np.float32),
        "alpha2": np.ascontiguousarray(alpha2, dtype=np.float32),
    }
    return [
        {"x": np.ascontiguousarray(x[i * B_PER_CORE:(i + 1) * B_PER_CORE]), **shared}
        for i in range(N_CORES)
    ]


def kernel(x, guide, pac_w, pac_b, alpha1, alpha2, conv_w, conv_b, **_unused):
    # guide is mathematically unused (adaptive kernel == exp(0) == 1)
    del guide
    in_maps = make_in_maps(x, pac_w, pac_b, alpha1, conv_w, conv_b, alpha2)
    # The first execution of a freshly loaded NEFF occasionally trips an
    # NRT_EXEC_UNIT_UNRECOVERABLE in the runtime (leftover device state from a
    # prior process).  Pause and retry, rebuilding the module so the runtime
    # reloads a fresh executable.
    import time as _time
    last_exc = None
    for attempt in range(3):
        try:
            nc = _get_nc()
            res = run_bass_kernel_spmd(
                nc, in_maps, list(range(N_CORES)), trace=_CACHE.get("trace", False)
            )
            break
        except Exception as exc:  # noqa: BLE001
            last_exc = exc
            _CACHE.pop(f"nc1_{S2_BF16}", None)
            _time.sleep(3.0 * (attempt + 1))
    else:
        raise last_exc
    _CACHE["last_result"] = res
    return np.concatenate([r["out"] for r in res.results], axis=0)


# revision 31
# speedup vs baseline: 1.2294x; 1.2294x over previous
"""Trainium2 Bass kernel for nn_DownBlock (PacConv1x1 -> PReLU -> Conv6x6s2 -> PReLU).

Math notes:
  - The PacConv2d adaptive kernel is exp(-0.5*||g-g||^2) == 1 exactly, so the
    guide tensor is mathematically unused: stage 1 is a plain 1x1 conv.
  - Stage 1: h[f,y,x] = prelu(sum_c pac_w[f,c] * x[c,y,x] + pac_b[f], alpha1)
  - Stage 2: 6x6 stride-2 conv with padding 2, + bias, prelu.

Implementation (per core, 2 of the 16 batch images, data-parallel over batch):
  - Stage 1 is a K=128 float32r matmul per 512-position tile; the rhs access
    pattern picks x-parity phases so the epilogue (bias+prelu, split between
    ACT and DVE) writes directly into the stage-2 input layout:
        Hx[(px, f), y+2, x//2 + 1]   (128 partitions, 132x66 image, zero halo)
  - Stage 2 (stride-2 6x6 conv) contracts (px, c) = 128 partitions per tap:
        out[o, i, j] = sum_{ky, n} Wp[ky,n][(px,c), o].T @ Hx[:, 2i+ky, j+n]
    = 18 accumulating K=128/M=64/N=512 matmuls per 8-row output block, then a
    fused Prelu epilogue and a DMA out.
  - S2_BF16 selects the stage-2 operand dtype: float32r (TF32-class accuracy,
    M=64 fills the whole PE array) or bfloat16 (two blocks run concurrently in
    the two column halves of the PE array via tile_position -> ~2x stage-2
    matmul throughput at bf16 input rounding).
"""
import numpy as np

import concourse.bacc as bacc
import concourse.mybir as mybir
from concourse.tile import TileContext
from concourse.bass_utils import run_bass_kernel_spmd
from concourse.masks import make_identity

F32 = mybir.dt.float32
F32R = mybir.dt.float32r
BF16 = mybir.dt.bfloat16
FP16 = mybir.dt.float16
ALU = mybir.AluOpType

N_CORES = 8
B_TOTAL = 16
B_PER_CORE = B_TOTAL // N_CORES  # 2
CIN = 128
F = 64   # intermediate / output channels
H = W = 128
HO = WO = 64
K = 6
# phase image: rows 0..131 (y+2), cols 0..65 (x//2+1), zero halo
PR = 132
PC = 66

S2_BF16 = True  # stage-2 matmul dtype: False -> float32r, True -> float16

_CACHE = {}


def _build(repeat=1, s2_bf16=S2_BF16):
    """Build the Bass module.  repeat>1 re-emits the main pipeline that many
    times back-to-back (bench-only: lets wall-clock slope isolate per-pass
    device time from the multi-ms axon dispatch overhead)."""
    nc = bacc.Bacc("TRN2", target_bir_lowering=False, debug=False)

    DT2 = FP16 if s2_bf16 else F32R
    # weights arrive pre-transposed / pre-cast from the host (numpy prep in
    # kernel()): pac_wT[c, f] = pac_w[f, c];  wp[(px, c), ky*3+n, o] =
    # conv_w[o, c, ky, 2n+px] in the stage-2 dtype.
    x = nc.declare_dram_parameter("x", [B_PER_CORE, CIN, H, W], DT2, isOutput=False)
    pac_wT_in = nc.declare_dram_parameter("pac_wT", [CIN, F], DT2, isOutput=False)
    pac_b = nc.declare_dram_parameter("pac_b", [F], F32, isOutput=False)
    alpha1 = nc.declare_dram_parameter("alpha1", [1], F32, isOutput=False)
    wp_in = nc.declare_dram_parameter("wp", [CIN, 18 * F], DT2, isOutput=False)
    conv_b = nc.declare_dram_parameter("conv_b", [F], F32, isOutput=False)
    alpha2 = nc.declare_dram_parameter("alpha2", [1], F32, isOutput=False)
    out = nc.declare_dram_parameter("out", [B_PER_CORE, F, HO, WO], F32, isOutput=True)

    PRELU = mybir.ActivationFunctionType.Prelu

    with TileContext(nc) as tc:
        with (
            tc.tile_pool(name="const", bufs=1) as const,
            tc.tile_pool(name="xin", bufs=12) as xin,
            tc.tile_pool(name="hx", bufs=1) as hxp,
            tc.tile_pool(name="ob", bufs=6) as obp,
            tc.tile_pool(name="dv", bufs=4) as dvp,
            tc.tile_pool(name="psA", bufs=4, space="PSUM") as psA,
        ):
            # ---------------- constants ----------------
            # per-partition scalars; stage-1 reads [0:64], the fp16 stage-2
            # epilogue reads all 128 (conv_b duplicated in both halves)
            b1 = const.tile([CIN, 1], F32)
            b2 = const.tile([CIN, 1], F32)
            a1 = const.tile([CIN, 1], F32)
            a2 = const.tile([CIN, 1], F32)
            nc.sync.dma_start(out=b1[0:64, :], in_=pac_b[:, None])
            nc.sync.dma_start(out=b1[64:128, :], in_=pac_b[:, None])
            nc.sync.dma_start(out=b2[0:64, :], in_=conv_b[:, None])
            nc.sync.dma_start(out=b2[64:128, :], in_=conv_b[:, None])
            nc.sync.dma_start(out=a1[:], in_=alpha1.broadcast_to([CIN, 1]))
            nc.sync.dma_start(out=a2[:], in_=alpha2.broadcast_to([CIN, 1]))

            pac_wT = const.tile([CIN, F], DT2)
            if DT2 == F32R:
                nc.sync.dma_start(out=pac_wT[:], in_=pac_wT_in[:].bitcast(F32R))
            else:
                nc.sync.dma_start(out=pac_wT[:], in_=pac_wT_in[:])
            wp = const.tile([CIN, 18 * F], DT2)
            if DT2 == F32R:
                nc.sync.dma_start(out=wp[:], in_=wp_in[:].bitcast(F32R))
            else:
                nc.sync.dma_start(out=wp[:], in_=wp_in[:])

            # ---------------- phase tensors + halo zeroing ----------------
            zrow = const.tile([CIN, 2 * PC], F32)
            nc.gpsimd.memset(zrow[:], 0.0)

            hx = [
                hxp.tile([CIN, PR * PC], DT2, tag=f"hx{b}", name=f"hx{b}")
                for b in range(B_PER_CORE)
            ]
            for b in range(B_PER_CORE):
                t = hx[b]
                # top rows 0..1, bottom rows 130..131
                nc.vector.tensor_copy(t[:, 0:2 * PC], zrow[:])
                nc.vector.tensor_copy(t[:, 130 * PC:132 * PC], zrow[:])
                # left col 0 and right col 65 stripes (132 rows each)
                nc.vector.tensor_copy(t[:, 0:PR * PC:PC], zrow[:, 0:PR])
                nc.vector.tensor_copy(t[:, 65:PR * PC:PC], zrow[:, 0:PR])

            # ---------------- main pipeline ----------------
            for b in [bb for _ in range(repeat) for bb in range(B_PER_CORE)]:
                hxb = hx[b].rearrange("p (r c) -> p r c", c=PC)
                for t in range(8):  # 16-row x chunks
                    xt = xin.tile([CIN, 16 * W], DT2, tag="xt")
                    xtv = xt[:].rearrange("p (r c) -> p r c", r=16)
                    # two half-chunk DMAs: h3=0 matmuls start after only 8 rows
                    nc.sync.dma_start(
                        out=xtv[:, 0:8],
                        in_=x[b, :, 16 * t:16 * t + 8, :]
                        if DT2 != F32R else x[b, :, 16 * t:16 * t + 8, :].bitcast(F32R),
                    )
                    nc.sync.dma_start(
                        out=xtv[:, 8:16],
                        in_=x[b, :, 16 * t + 8:16 * t + 16, :]
                        if DT2 != F32R else x[b, :, 16 * t + 8:16 * t + 16, :].bitcast(F32R),
                    )
                    for h3 in range(2):  # 8-row halves
                        r0 = 16 * t + 8 * h3 + 2
                        if s2_bf16:
                            # both x-parity halves into one bank (each matmul is
                            # a closed start+stop group, so no pending-group
                            # conflict), then ONE full-width prelu epilogue.
                            ps = psA.tile([CIN, 8, 64], F32, tag="s1", bufs=4,
                                          name="ps")
                            for px in range(2):
                                nc.tensor.matmul(
                                    ps[px * 64:(px + 1) * 64],
                                    pac_wT[:],
                                    xtv[:, 8 * h3:8 * h3 + 8, px::2],
                                    start=True, stop=True,
                                    tile_position=(0, px * 64),
                                )
                            dst = hxb[0:128, r0:r0 + 8, 1:65]
                            if h3 == 0 and t % 2 == 0:
                                # keep some epilogue work on DVE for balance
                                t1 = dvp.tile([CIN, 8, 64], F32, tag="dv1", name="t1")
                                t2 = dvp.tile([CIN, 8, 64], F32, tag="dv2", name="t2")
                                nc.vector.tensor_scalar(
                                    t1[:], ps[:], b1[:], 0.0, ALU.add, ALU.max)
                                nc.vector.tensor_scalar(
                                    t2[:], ps[:], b1[:], 0.0, ALU.add, ALU.min)
                                nc.vector.scalar_tensor_tensor(
                                    dst, t2[:], a1[:], t1[:], ALU.mult, ALU.add)
                            else:
                                nc.scalar.activation(
                                    dst, ps[:], PRELU,
                                    bias=b1[:], scale=1.0, alpha=a1[:],
                                )
                        else:
                            for px in range(2):
                                ps = psA.tile([F, 8, 64], F32, tag="s1")
                                nc.tensor.matmul(
                                    ps[:],
                                    pac_wT[:],
                                    xtv[:, 8 * h3:8 * h3 + 8, px::2],
                                    start=True, stop=True,
                                )
                                dst = hxb[px * 64:(px + 1) * 64, r0:r0 + 8, 1:65]
                                nc.scalar.activation(
                                    dst, ps[:], PRELU,
                                    bias=b1[0:64, :], scale=1.0, alpha=a1[0:64, :],
                                )
                    if s2_bf16:
                        # paired blocks: pair s ready after chunk 2s+2
                        if t >= 2 and t % 2 == 0:
                            _s2_pair(nc, psA, obp, hxb, wp, b2, a2, out, b, (t - 2) // 2)
                    else:
                        if t >= 1:
                            _s2_single(nc, psA, obp, hxb, wp, b2, a2, out, b, t - 1)
                if s2_bf16:
                    _s2_pair(nc, psA, obp, hxb, wp, b2, a2, out, b, 3)
                else:
                    _s2_single(nc, psA, obp, hxb, wp, b2, a2, out, b, 7)

    nc.compile()
    return nc


def _s2_single(nc, psA, obp, hxb, wp, b2, a2, out, b, ib):
    """float32r: 18 accumulating taps -> prelu -> dma, output rows [8ib, 8ib+8)."""
    PRELU = mybir.ActivationFunctionType.Prelu
    ps = psA.tile([F, 8, 64], F32, tag="s2", name="ps")
    for ky in range(K):
        for n in range(3):
            t18 = ky * 3 + n
            r0 = 16 * ib + ky
            rhs = hxb[:, r0:min(r0 + 16, PR):2, n:n + 64]
            nc.tensor.matmul(
                ps[:], wp[:, t18 * F:(t18 + 1) * F], rhs,
                start=(t18 == 0), stop=(t18 == 17),
            )
    ot = obp.tile([F, 8, 64], F32, tag="ot", name="ot")
    nc.scalar.activation(ot[:], ps[:], PRELU, bias=b2[0:64, :], scale=1.0,
                         alpha=a2[0:64, :])
    nc.sync.dma_start(out=out[b, :, 8 * ib:8 * ib + 8, :], in_=ot[:])


def _s2_pair(nc, psA, obp, hxb, wp, b2, a2, out, b, s):
    """bf16/fp16: blocks 2s and 2s+1 run concurrently in the two column halves
    of the PE array (tile_position).  Each half accumulates in its OWN psum
    bank — two interleaved accumulation groups in one bank are illegal (the
    sequencer tracks pending groups per bank zero-region)."""
    PRELU = mybir.ActivationFunctionType.Prelu
    psh = [
        psA.tile([CIN, 8, 64], F32, tag=f"s2{'ab'[h]}", name=f"ps{h}", bufs=2)
        for h in range(2)
    ]
    for ky in range(K):
        for n in range(3):
            t18 = ky * 3 + n
            for half in range(2):
                ib = 2 * s + half
                r0 = 16 * ib + ky
                rhs = hxb[:, r0:min(r0 + 16, PR):2, n:n + 64]
                nc.tensor.matmul(
                    psh[half][half * 64:(half + 1) * 64],
                    wp[:, t18 * F:(t18 + 1) * F],
                    rhs, start=(t18 == 0), stop=(t18 == 17),
                    tile_position=(0, half * 64),
                )
    ot = obp.tile([CIN, 8, 64], F32, tag="ot", name="ot")
    for half in range(2):
        sl = slice(half * 64, (half + 1) * 64)
        nc.scalar.activation(ot[sl], psh[half][sl], PRELU,
                             bias=b2[sl, :], scale=1.0, alpha=a2[sl, :])
    for half in range(2):
        ib = 2 * s + half
        nc.sync.dma_start(
            out=out[b, :, 8 * ib:8 * ib + 8, :],
            in_=ot[half * 64:(half + 1) * 64],
        )


def _get_nc(repeat=1, s2_bf16=S2_BF16):
    key = f"nc{repeat}_{s2_bf16}"
    if key not in _CACHE:
        _CACHE[key] = _build(repeat, s2_bf16)
    return _CACHE[key]


def make_in_maps(x, pac_w, pac_b, alpha1, conv_w, conv_b, alpha2):
    """Host-side prep: shard x over cores, pre-transpose/cast the weights."""
    np_dtx = np.float16 if S2_BF16 else np.float32
    x = np.ascontiguousarray(np.asarray(x, dtype=np.float32), dtype=np_dtx)
    pac_w = np.asarray(pac_w, dtype=np.float32).reshape(F, CIN)
    conv_w4 = np.asarray(conv_w, dtype=np.float32).reshape(F, F, K, K)
    np_dt2 = np.float16 if S2_BF16 else np.float32
    wp_np = np.zeros((CIN, 18 * F), dtype=np_dt2)
    for ky in range(K):
        for n in range(3):
            t18 = ky * 3 + n
            for px in range(2):
                # wp[(px, c), t18*F + o] = conv_w[o, c, ky, 2n+px]
                wp_np[px * F:(px + 1) * F, t18 * F:(t18 + 1) * F] = (
                    conv_w4[:, :, ky, 2 * n + px].T.astype(np_dt2)
                )
    shared = {
        "pac_wT": np.ascontiguousarray(pac_w.T.astype(np_dtx)),
        "pac_b": np.ascontiguousarray(pac_b, dtype=np.float32),
        "alpha1": np.ascontiguousarray(alpha1, dtype=np.float32),
        "wp": wp_np,
        "conv_b": np.ascontiguousarray(conv_b, dtype=np.float32),
        "alpha2": np.ascontiguousarray(alpha2, dtype=np.float32),
    }
    return [
        {"x": np.ascontiguousarray(x[i * B_PER_CORE:(i + 1) * B_PER_CORE]), **shared}
        for i in range(N_CORES)
    ]


def kernel(x, guide, pac_w, pac_b, alpha1, alpha2, conv_w, conv_b, **_unused):
    # guide is mathematically unused (adaptive kernel == exp(0) == 1)
    del guide
    in_maps = make_in_maps(x, pac_w, pac_b, alpha1, conv_w, conv_b, alpha2)
    # The first execution of a freshly loaded NEFF occasionally trips an
    # NRT_EXEC_UNIT_UNRECOVERABLE in the runtime (leftover device state from a
    # prior process).  Pause and retry, rebuilding the module so the runtime
    # reloads a fresh executable.
    import time as _time
    last_exc = None
    for attempt in range(3):
        try:
            nc = _get_nc()
            res = run_bass_kernel_spmd(
                nc, in_maps, list(range(N_CORES)), trace=_CACHE.get("trace", False)
            )
            break
        except Exception as exc:  # noqa: BLE001
            last_exc = exc
            _CACHE.pop(f"nc1_{S2_BF16}", None)
            _time.sleep(3.0 * (attempt + 1))
    else:
        raise last_exc
    _CACHE["last_result"] = res
    return np.concatenate([r["out"] for r in res.results], axis=0)


# revision 32
# speedup vs baseline: 1.4614x; 1.1887x over previous
"""Trainium2 Bass kernel for nn_DownBlock (PacConv1x1 -> PReLU -> Conv6x6s2 -> PReLU).

Math notes:
  - The PacConv2d adaptive kernel is exp(-0.5*||g-g||^2) == 1 exactly, so the
    guide tensor is mathematically unused: stage 1 is a plain 1x1 conv.
  - Stage 1: h[f,y,x] = prelu(sum_c pac_w[f,c] * x[c,y,x] + pac_b[f], alpha1)
  - Stage 2: 6x6 stride-2 conv with padding 2, + bias, prelu.

Implementation (per core, 2 of the 16 batch images, data-parallel over batch):
  - Stage 1 is a K=128 float32r matmul per 512-position tile; the rhs access
    pattern picks x-parity phases so the epilogue (bias+prelu, split between
    ACT and DVE) writes directly into the stage-2 input layout:
        Hx[(px, f), y+2, x//2 + 1]   (128 partitions, 132x66 image, zero halo)
  - Stage 2 (stride-2 6x6 conv) contracts (px, c) = 128 partitions per tap:
        out[o, i, j] = sum_{ky, n} Wp[ky,n][(px,c), o].T @ Hx[:, 2i+ky, j+n]
    = 18 accumulating K=128/M=64/N=512 matmuls per 8-row output block, then a
    fused Prelu epilogue and a DMA out.
  - S2_BF16 selects the stage-2 operand dtype: float32r (TF32-class accuracy,
    M=64 fills the whole PE array) or bfloat16 (two blocks run concurrently in
    the two column halves of the PE array via tile_position -> ~2x stage-2
    matmul throughput at bf16 input rounding).
"""
import numpy as np

import concourse.bacc as bacc
import concourse.mybir as mybir
from concourse.tile import TileContext
from concourse.bass_utils import run_bass_kernel_spmd
from concourse.masks import make_identity

F32 = mybir.dt.float32
F32R = mybir.dt.float32r
BF16 = mybir.dt.bfloat16
FP16 = mybir.dt.float16
ALU = mybir.AluOpType

N_CORES = 8
B_TOTAL = 16
B_PER_CORE = B_TOTAL // N_CORES  # 2
CIN = 128
F = 64   # intermediate / output channels
H = W = 128
HO = WO = 64
K = 6
# phase image: rows 0..131 (y+2), cols 0..65 (x//2+1), zero halo
PR = 132
PC = 66

S2_BF16 = True  # stage-2 matmul dtype: False -> float32r, True -> float16

_CACHE = {}


def _build(repeat=1, s2_bf16=S2_BF16):
    """Build the Bass module.  repeat>1 re-emits the main pipeline that many
    times back-to-back (bench-only: lets wall-clock slope isolate per-pass
    device time from the multi-ms axon dispatch overhead)."""
    nc = bacc.Bacc("TRN2", target_bir_lowering=False, debug=False)

    DT2 = FP16 if s2_bf16 else F32R
    # weights arrive pre-transposed / pre-cast from the host (numpy prep in
    # kernel()): pac_wT[c, f] = pac_w[f, c];  wp[(px, c), ky*3+n, o] =
    # conv_w[o, c, ky, 2n+px] in the stage-2 dtype.
    x = nc.declare_dram_parameter("x", [B_PER_CORE, CIN, H, W], DT2, isOutput=False)
    pac_wT_in = nc.declare_dram_parameter("pac_wT", [CIN, F], DT2, isOutput=False)
    # consts[:, 0]=pac_b (dup both halves), 1=conv_b (dup), 2=alpha1, 3=alpha2
    consts_in = nc.declare_dram_parameter("consts", [CIN, 4], F32, isOutput=False)
    wp_in = nc.declare_dram_parameter("wp", [CIN, 18 * F], DT2, isOutput=False)
    out = nc.declare_dram_parameter("out", [B_PER_CORE, F, HO, WO], F32, isOutput=True)

    PRELU = mybir.ActivationFunctionType.Prelu

    with TileContext(nc) as tc:
        with (
            tc.tile_pool(name="const", bufs=1) as const,
            tc.tile_pool(name="xin", bufs=12) as xin,
            tc.tile_pool(name="hx", bufs=1) as hxp,
            tc.tile_pool(name="ob", bufs=6) as obp,
            tc.tile_pool(name="dv", bufs=4) as dvp,
            tc.tile_pool(name="psA", bufs=4, space="PSUM") as psA,
        ):
            # ---------------- constants ----------------
            # per-partition scalars, one DMA (each extra const DMA costs ~1us
            # of first-byte latency ahead of the first matmul)
            cs = const.tile([CIN, 4], F32)
            nc.sync.dma_start(out=cs[:], in_=consts_in[:])
            b1 = cs[:, 0:1]
            b2 = cs[:, 1:2]
            a1 = cs[:, 2:3]
            a2 = cs[:, 3:4]

            pac_wT = const.tile([CIN, F], DT2)
            if DT2 == F32R:
                nc.sync.dma_start(out=pac_wT[:], in_=pac_wT_in[:].bitcast(F32R))
            else:
                nc.sync.dma_start(out=pac_wT[:], in_=pac_wT_in[:])
            wp = const.tile([CIN, 18 * F], DT2)
            if DT2 == F32R:
                nc.sync.dma_start(out=wp[:], in_=wp_in[:].bitcast(F32R))
            else:
                nc.sync.dma_start(out=wp[:], in_=wp_in[:])

            # ---------------- phase tensors + halo zeroing ----------------
            zrow = const.tile([CIN, 2 * PC], F32)
            nc.gpsimd.memset(zrow[:], 0.0)

            hx = [
                hxp.tile([CIN, PR * PC], DT2, tag=f"hx{b}", name=f"hx{b}")
                for b in range(B_PER_CORE)
            ]
            for b in range(B_PER_CORE):
                t = hx[b]
                # top rows 0..1, bottom rows 130..131
                nc.vector.tensor_copy(t[:, 0:2 * PC], zrow[:])
                nc.vector.tensor_copy(t[:, 130 * PC:132 * PC], zrow[:])
                # left col 0 and right col 65 stripes (132 rows each)
                nc.vector.tensor_copy(t[:, 0:PR * PC:PC], zrow[:, 0:PR])
                nc.vector.tensor_copy(t[:, 65:PR * PC:PC], zrow[:, 0:PR])

            # ---------------- main pipeline ----------------
            for b in [bb for _ in range(repeat) for bb in range(B_PER_CORE)]:
                hxb = hx[b].rearrange("p (r c) -> p r c", c=PC)
                for t in range(8):  # 16-row x chunks
                    xt = xin.tile([CIN, 16 * W], DT2, tag="xt")
                    xtv = xt[:].rearrange("p (r c) -> p r c", r=16)
                    # two half-chunk DMAs: h3=0 matmuls start after only 8 rows
                    nc.sync.dma_start(
                        out=xtv[:, 0:8],
                        in_=x[b, :, 16 * t:16 * t + 8, :]
                        if DT2 != F32R else x[b, :, 16 * t:16 * t + 8, :].bitcast(F32R),
                    )
                    nc.sync.dma_start(
                        out=xtv[:, 8:16],
                        in_=x[b, :, 16 * t + 8:16 * t + 16, :]
                        if DT2 != F32R else x[b, :, 16 * t + 8:16 * t + 16, :].bitcast(F32R),
                    )
                    for h3 in range(2):  # 8-row halves
                        r0 = 16 * t + 8 * h3 + 2
                        if s2_bf16:
                            # both x-parity halves into one bank (each matmul is
                            # a closed start+stop group, so no pending-group
                            # conflict), then ONE full-width prelu epilogue.
                            ps = psA.tile([CIN, 8, 64], F32, tag="s1", bufs=4,
                                          name="ps")
                            for px in range(2):
                                nc.tensor.matmul(
                                    ps[px * 64:(px + 1) * 64],
                                    pac_wT[:],
                                    xtv[:, 8 * h3:8 * h3 + 8, px::2],
                                    start=True, stop=True,
                                    tile_position=(0, px * 64),
                                )
                            dst = hxb[0:128, r0:r0 + 8, 1:65]
                            if h3 == 0 and t % 2 == 0:
                                # keep some epilogue work on DVE for balance
                                t1 = dvp.tile([CIN, 8, 64], F32, tag="dv1", name="t1")
                                t2 = dvp.tile([CIN, 8, 64], F32, tag="dv2", name="t2")
                                nc.vector.tensor_scalar(
                                    t1[:], ps[:], b1, 0.0, ALU.add, ALU.max)
                                nc.vector.tensor_scalar(
                                    t2[:], ps[:], b1, 0.0, ALU.add, ALU.min)
                                nc.vector.scalar_tensor_tensor(
                                    dst, t2[:], a1, t1[:], ALU.mult, ALU.add)
                            else:
                                nc.scalar.activation(
                                    dst, ps[:], PRELU,
                                    bias=b1, scale=1.0, alpha=a1,
                                )
                        else:
                            for px in range(2):
                                ps = psA.tile([F, 8, 64], F32, tag="s1")
                                nc.tensor.matmul(
                                    ps[:],
                                    pac_wT[:],
                                    xtv[:, 8 * h3:8 * h3 + 8, px::2],
                                    start=True, stop=True,
                                )
                                dst = hxb[px * 64:(px + 1) * 64, r0:r0 + 8, 1:65]
                                nc.scalar.activation(
                                    dst, ps[:], PRELU,
                                    bias=b1[0:64], scale=1.0, alpha=a1[0:64],
                                )
                    if s2_bf16:
                        # paired blocks: pair s ready after chunk 2s+2
                        if t >= 2 and t % 2 == 0:
                            _s2_pair(nc, psA, obp, hxb, wp, b2, a2, out, b, (t - 2) // 2)
                    else:
                        if t >= 1:
                            _s2_single(nc, psA, obp, hxb, wp, b2, a2, out, b, t - 1)
                if s2_bf16:
                    _s2_pair(nc, psA, obp, hxb, wp, b2, a2, out, b, 3)
                else:
                    _s2_single(nc, psA, obp, hxb, wp, b2, a2, out, b, 7)

    nc.compile()
    return nc


def _s2_single(nc, psA, obp, hxb, wp, b2, a2, out, b, ib):
    """float32r: 18 accumulating taps -> prelu -> dma, output rows [8ib, 8ib+8)."""
    PRELU = mybir.ActivationFunctionType.Prelu
    ps = psA.tile([F, 8, 64], F32, tag="s2", name="ps")
    for ky in range(K):
        for n in range(3):
            t18 = ky * 3 + n
            r0 = 16 * ib + ky
            rhs = hxb[:, r0:min(r0 + 16, PR):2, n:n + 64]
            nc.tensor.matmul(
                ps[:], wp[:, t18 * F:(t18 + 1) * F], rhs,
                start=(t18 == 0), stop=(t18 == 17),
            )
    ot = obp.tile([F, 8, 64], F32, tag="ot", name="ot")
    nc.scalar.activation(ot[:], ps[:], PRELU, bias=b2[0:64], scale=1.0,
                         alpha=a2[0:64])
    nc.sync.dma_start(out=out[b, :, 8 * ib:8 * ib + 8, :], in_=ot[:])


def _s2_pair(nc, psA, obp, hxb, wp, b2, a2, out, b, s):
    """bf16/fp16: blocks 2s and 2s+1 run concurrently in the two column halves
    of the PE array (tile_position).  Each half accumulates in its OWN psum
    bank — two interleaved accumulation groups in one bank are illegal (the
    sequencer tracks pending groups per bank zero-region)."""
    PRELU = mybir.ActivationFunctionType.Prelu
    psh = [
        psA.tile([CIN, 8, 64], F32, tag=f"s2{'ab'[h]}", name=f"ps{h}", bufs=2)
        for h in range(2)
    ]
    for ky in range(K):
        for n in range(3):
            t18 = ky * 3 + n
            for half in range(2):
                ib = 2 * s + half
                r0 = 16 * ib + ky
                rhs = hxb[:, r0:min(r0 + 16, PR):2, n:n + 64]
                nc.tensor.matmul(
                    psh[half][half * 64:(half + 1) * 64],
                    wp[:, t18 * F:(t18 + 1) * F],
                    rhs, start=(t18 == 0), stop=(t18 == 17),
                    tile_position=(0, half * 64),
                )
    ot = obp.tile([CIN, 8, 64], F32, tag="ot", name="ot")
    for half in range(2):
        sl = slice(half * 64, (half + 1) * 64)
        nc.scalar.activation(ot[sl], psh[half][sl], PRELU,
                             bias=b2[sl], scale=1.0, alpha=a2[sl])
    for half in range(2):
        ib = 2 * s + half
        nc.sync.dma_start(
            out=out[b, :, 8 * ib:8 * ib + 8, :],
            in_=ot[half * 64:(half + 1) * 64],
        )


def _get_nc(repeat=1, s2_bf16=S2_BF16):
    key = f"nc{repeat}_{s2_bf16}"
    if key not in _CACHE:
        _CACHE[key] = _build(repeat, s2_bf16)
    return _CACHE[key]


def make_in_maps(x, pac_w, pac_b, alpha1, conv_w, conv_b, alpha2):
    """Host-side prep: shard x over cores, pre-transpose/cast the weights."""
    np_dtx = np.float16 if S2_BF16 else np.float32
    x = np.ascontiguousarray(np.asarray(x, dtype=np.float32), dtype=np_dtx)
    pac_w = np.asarray(pac_w, dtype=np.float32).reshape(F, CIN)
    conv_w4 = np.asarray(conv_w, dtype=np.float32).reshape(F, F, K, K)
    np_dt2 = np.float16 if S2_BF16 else np.float32
    wp_np = np.zeros((CIN, 18 * F), dtype=np_dt2)
    for ky in range(K):
        for n in range(3):
            t18 = ky * 3 + n
            for px in range(2):
                # wp[(px, c), t18*F + o] = conv_w[o, c, ky, 2n+px]
                wp_np[px * F:(px + 1) * F, t18 * F:(t18 + 1) * F] = (
                    conv_w4[:, :, ky, 2 * n + px].T.astype(np_dt2)
                )
    consts = np.zeros((CIN, 4), dtype=np.float32)
    consts[0:64, 0] = consts[64:128, 0] = np.asarray(pac_b, dtype=np.float32)
    consts[0:64, 1] = consts[64:128, 1] = np.asarray(conv_b, dtype=np.float32)
    consts[:, 2] = np.float32(np.asarray(alpha1).reshape(-1)[0])
    consts[:, 3] = np.float32(np.asarray(alpha2).reshape(-1)[0])
    shared = {
        "pac_wT": np.ascontiguousarray(pac_w.T.astype(np_dtx)),
        "consts": consts,
        "wp": wp_np,
    }
    return [
        {"x": np.ascontiguousarray(x[i * B_PER_CORE:(i + 1) * B_PER_CORE]), **shared}
        for i in range(N_CORES)
    ]


def kernel(x, guide, pac_w, pac_b, alpha1, alpha2, conv_w, conv_b, **_unused):
    # guide is mathematically unused (adaptive kernel == exp(0) == 1)
    del guide
    in_maps = make_in_maps(x, pac_w, pac_b, alpha1, conv_w, conv_b, alpha2)
    # The first execution of a freshly loaded NEFF occasionally trips an
    # NRT_EXEC_UNIT_UNRECOVERABLE in the runtime (leftover device state from a
    # prior process).  Pause and retry, rebuilding the module so the runtime
    # reloads a fresh executable.
    import time as _time
    last_exc = None
    for attempt in range(3):
        try:
            nc = _get_nc()
            res = run_bass_kernel_spmd(
                nc, in_maps, list(range(N_CORES)), trace=_CACHE.get("trace", False)
            )
            break
        except Exception as exc:  # noqa: BLE001
            last_exc = exc
            _CACHE.pop(f"nc1_{S2_BF16}", None)
            _time.sleep(3.0 * (attempt + 1))
    else:
        raise last_exc
    _CACHE["last_result"] = res
    return np.concatenate([r["out"] for r in res.results], axis=0)
